# revision 25
# baseline (speedup 1.0000x reference)
"""Multi-head attention (B=2, N=2048, D=1024, H=16) on 8 NeuronCores.

Sharding: data-parallel over batch (4 cores per batch element), tensor-
parallel over heads (4 heads per core). Each core computes, for its
(batch, head-group):
    QT/KT = (x Wq^T + bq)^T, V = x Wv^T + bv          (QKV projection)
    S^T   = K Q^T  (per head, k on partitions, q on free)
    P^T   = exp(S^T / 8)                               (no max-subtract:
            scores are O(+-20), exp is safe in fp32)
    [O^T; r] = [V | 1]^T-augmented PV matmul           (row 64 = softmax
            denominators, accumulated alongside O^T)
    O^T  *= 1/r  (per-q normalize)
    Ypart = O @ Wo_slice^T                             (partial over heads)
Host sums the 4 partials per batch and adds out_b.

All weights/x are pre-transposed on the host so the device never
transposes anything.
"""

import numpy as np

import concourse.bass as bass
import concourse.tile as tile
from concourse import bacc, mybir
from concourse.bass_utils import run_bass_kernel_spmd

B, N, D = 2, 2048, 1024
H, HD = 16, 64
NCORES = 8
CPB = 4            # cores per batch element
HPC = H // CPB     # heads per core = 4
DL = HPC * HD      # local (per-core) model dim = 256
FP32 = mybir.dt.float32

# matmul input mode: "f32" | "f32r" | "bf16"
MODE = "f32r"


def _md(mode):
    return {
        "f32": mybir.dt.float32,
        "f32r": mybir.dt.float32r,
        "bf16": mybir.dt.bfloat16,
    }[mode]


def build_nc(mode=MODE):
    md_store = _md(mode)

    nc = bacc.Bacc(
        "TRN2", target_bir_lowering=False, debug=False, num_devices=NCORES
    )

    xt_d = nc.dram_tensor("xt", [D, N], md_store, kind="ExternalInput").ap()
    wqt_d = nc.dram_tensor("wqt", [D, DL], md_store, kind="ExternalInput").ap()
    wkt_d = nc.dram_tensor("wkt", [D, DL], md_store, kind="ExternalInput").ap()
    wvt_d = nc.dram_tensor("wvt", [D, DL], md_store, kind="ExternalInput").ap()
    wot_d = nc.dram_tensor("wot", [DL, D], md_store, kind="ExternalInput").ap()
    bq_d = nc.dram_tensor("bq", [128, 2], FP32, kind="ExternalInput").ap()
    bk_d = nc.dram_tensor("bk", [128, 2], FP32, kind="ExternalInput").ap()
    bv_d = nc.dram_tensor("bv", [1, DL], md_store, kind="ExternalInput").ap()
    y_d = nc.dram_tensor("y", [N, D], FP32, kind="ExternalOutput").ap()

    with tile.TileContext(nc) as tc:
        with (
            tc.tile_pool(name="const", bufs=1) as const,
            tc.tile_pool(name="pt", bufs=2 if mode != "bf16" else 4) as ptp,
            tc.tile_pool(name="ysb", bufs=2) as yp,
            tc.tile_pool(name="small", bufs=2) as smallp,
            tc.tile_pool(name="sps", bufs=2, space="PSUM") as spsum,
            tc.tile_pool(name="ops", bufs=4, space="PSUM") as opsum,
        ):
            # ---- resident SBUF tensors ----
            xt_sb = const.tile([128, 8, N], md_store)
            wqt_sb = const.tile([128, 8, DL], md_store)
            wkt_sb = const.tile([128, 8, DL], md_store)
            wvt_sb = const.tile([128, 8, DL], md_store)
            wot_sb = const.tile([128, 2, D], md_store)
            bq_sb = const.tile([128, 2], FP32)
            bk_sb = const.tile([128, 2], FP32)
            bv_sb = const.tile([1, DL], md_store)
            ones_sb = const.tile([1, 128], md_store)
            ones64 = const.tile([128, HD], FP32)
            qt_sb = const.tile([128, 2, N], md_store)
            kt_sb = const.tile([128, 2, N], md_store)
            v_sb = const.tile([128, 16, HPC, HD + 1], md_store)
            ot_sb = const.tile([128, 2, N], md_store)
            # Softmax denominator rows parked at partition 32*h (DVE APs may
            # only start at partition multiples of 32), q-block along free.
            r_all = const.tile([128, N], FP32)

            nc.sync.dma_start(xt_sb[:], xt_d.rearrange("(o p) n -> p o n", p=128))
            nc.sync.dma_start(wqt_sb[:], wqt_d.rearrange("(o p) m -> p o m", p=128))
            nc.sync.dma_start(wkt_sb[:], wkt_d.rearrange("(o p) m -> p o m", p=128))
            nc.sync.dma_start(wvt_sb[:], wvt_d.rearrange("(o p) m -> p o m", p=128))
            nc.sync.dma_start(wot_sb[:], wot_d.rearrange("(o p) m -> p o m", p=128))
            nc.sync.dma_start(bq_sb[:], bq_d)
            nc.sync.dma_start(bk_sb[:], bk_d)
            nc.sync.dma_start(bv_sb[:], bv_d)
            nc.vector.memset(r_all[:], 1.0)
            nc.vector.memset(ones64[:], 1.0)
            if mode == "f32r":
                # memset doesn't pass the ISA check for float32r; write the
                # 1.0f bit pattern through a uint32 view instead.
                nc.vector.memset(ones_sb[:].bitcast(mybir.dt.uint32), 0x3F800000)
                nc.vector.memset(
                    v_sb[:, :, :, HD : HD + 1].bitcast(mybir.dt.uint32), 0x3F800000
                )
            else:
                nc.vector.memset(ones_sb[:], 1.0)
                nc.vector.memset(v_sb[:, :, :, HD : HD + 1], 1.0)

            Ident = mybir.ActivationFunctionType.Identity

            # ---- QKV projection ----
            # QT/KT: [dl on partitions (2 groups), n on free]
            for w_sb, b_sb, dst in ((wqt_sb, bq_sb, qt_sb), (wkt_sb, bk_sb, kt_sb)):
                for g in range(2):
                    for qs in range(4):
                        ps = spsum.tile([128, 1024], FP32, tag="sps")
                        acc = ps[:, :512]
                        for cc in range(8):
                            nc.tensor.matmul(
                                acc,
                                lhsT=(w_sb[:, cc, g * 128 : (g + 1) * 128]),
                                rhs=(xt_sb[:, cc, qs * 512 : (qs + 1) * 512]),
                                start=(cc == 0),
                                stop=(cc == 7),
                            )
                        nc.scalar.activation(
                            out=dst[:, g, qs * 512 : (qs + 1) * 512],
                            in_=acc,
                            func=Ident,
                            bias=b_sb[:, g : g + 1],
                        )
            # V: natural [n on partitions, dl on free], bias via ones-row matmul
            for nt in range(16):
                ps = spsum.tile([128, 1024], FP32, tag="sps")
                acc = ps[:, :DL]
                for cc in range(8):
                    nc.tensor.matmul(
                        acc,
                        lhsT=(xt_sb[:, cc, nt * 128 : (nt + 1) * 128]),
                        rhs=(wvt_sb[:, cc, :]),
                        start=(cc == 0),
                        stop=False,
                    )
                nc.tensor.matmul(
                    acc,
                    lhsT=ones_sb[:],
                    rhs=bv_sb[:],
                    start=False,
                    stop=True,
                )
                for h in range(HPC):
                    nc.vector.tensor_copy(
                        out=v_sb[:, nt, h, 0:HD], in_=acc[:, h * HD : (h + 1) * HD]
                    )

            # ---- attention, head PAIRS interleaved ----
            # The two heads of a pair have K=64 contractions at partition
            # bases 0 and 64: their S^T matmuls land in different PE row
            # groups and run concurrently, keeping the full array active
            # (a half-idle array trips the HAM throttle to 1.2 GHz).
            for qh in range(2):
                for g in range(2):
                    o_ps = [
                        opsum.tile([128, 512], FP32, tag="ops", name=f"ops_{g}_{qh}_{i}")
                        for i in range(4)  # (h0,j0) (h1,j0) (h0,j1) (h1,j1)
                    ]
                    for kt in range(16):
                        pt = ptp.tile([128, N], md_store, tag="pt")
                        s_pair = [
                            spsum.tile([128, 1024], FP32, tag="sps", name=f"s_{g}_{qh}_{hh}")
                            for hh in range(2)
                        ]
                        for j in range(2):
                            for hh in range(2):
                                po = hh * HD
                                qs = qh * 1024 + j * 512
                                nc.tensor.matmul(
                                    s_pair[hh][:, j * 512 : (j + 1) * 512],
                                    lhsT=kt_sb[
                                        po : po + HD, g, kt * 128 : (kt + 1) * 128
                                    ],
                                    rhs=(qt_sb[po : po + HD, g, qs : qs + 512]),
                                    start=True,
                                    stop=True,
                                )
                        for hh in range(2):
                            nc.scalar.activation(
                                out=pt[:, hh * 1024 : (hh + 1) * 1024],
                                in_=s_pair[hh][:],
                                func=mybir.ActivationFunctionType.Exp,
                                scale=0.125,
                            )
                        for j in range(2):
                            for hh in range(2):
                                h = 2 * g + hh
                                nc.tensor.matmul(
                                    o_ps[2 * j + hh][:65, :],
                                    lhsT=(v_sb[:, kt, h, :]),
                                    rhs=(pt[:, hh * 1024 + j * 512 :][:, :512]),
                                    start=(kt == 0),
                                    stop=(kt == 15),
                                )
                    # Drain accumulators fast (unnormalized) so the next
                    # pass can reuse the PSUM banks; normalization batched.
                    for j in range(2):
                        for hh in range(2):
                            h = 2 * g + hh
                            po = hh * HD
                            qg = qh * 2 + j
                            nc.vector.tensor_copy(
                                out=r_all[
                                    32 * h : 32 * h + 1, qg * 512 : (qg + 1) * 512
                                ],
                                in_=o_ps[2 * j + hh][64:65, :],
                            )
                            nc.vector.tensor_copy(
                                out=ot_sb[po : po + HD, g, qg * 512 : (qg + 1) * 512],
                                in_=o_ps[2 * j + hh][0:HD, :],
                            )

                # ---- normalize + output-projection for this q-half ----
                # Runs as soon as both head-pairs of the q-half have drained,
                # overlapping the next q-half's attention on PE/DVE bubbles.
                nc.vector.reciprocal(
                    r_all[:, qh * 1024 : (qh + 1) * 1024],
                    r_all[:, qh * 1024 : (qh + 1) * 1024],
                )
                for h in range(HPC):
                    g, po = h // 2, (h % 2) * HD
                    for j in range(2):
                        qg = qh * 2 + j
                        # Broadcast the reciprocal row across 64 partitions on
                        # the PE (ones[1,64].T @ r[1,512]); gpsimd
                        # partition_broadcast mis-reads nonzero base
                        # partitions on HW.
                        bc_ps = spsum.tile([128, 1024], FP32, tag="sps")
                        nc.tensor.matmul(
                            bc_ps[:HD, :512],
                            lhsT=ones64[32 * h : 32 * h + 1, :],
                            rhs=r_all[32 * h : 32 * h + 1, qg * 512 : (qg + 1) * 512],
                            start=True,
                            stop=True,
                            # auto-derive caps at base partition 64
                            tile_position=(32 * h, 0) if h == 3 else None,
                        )
                        sl = ot_sb[po : po + HD, g, qg * 512 : (qg + 1) * 512]
                        nc.vector.tensor_tensor(
                            out=sl,
                            in0=sl,
                            in1=bc_ps[0:HD, :512],
                            op=mybir.AluOpType.mult,
                        )
                for nt in range(qh * 8, qh * 8 + 8):
                    y_sb = yp.tile([128, D], FP32, tag="y")
                    for jg in range(2):
                        yps = opsum.tile([128, 512], FP32, tag="ops")
                        for dg in range(2):
                            nc.tensor.matmul(
                                yps[:],
                                lhsT=(ot_sb[:, dg, nt * 128 : (nt + 1) * 128]),
                                rhs=(wot_sb[:, dg, jg * 512 : (jg + 1) * 512]),
                                start=(dg == 0),
                                stop=(dg == 1),
                            )
                        nc.vector.tensor_copy(
                            out=y_sb[:, jg * 512 : (jg + 1) * 512], in_=yps[:]
                        )
                    nc.sync.dma_start(y_d[nt * 128 : (nt + 1) * 128, :], y_sb[:])

    nc.compile()
    return nc


def make_in_maps(x, qkv_w, qkv_b, mode=MODE):
    """Host-side shard + pre-transpose. Core c -> batch c//CPB, heads
    [(c%CPB)*HPC, ...+HPC)."""
    import ml_dtypes

    md_np = ml_dtypes.bfloat16 if mode == "bf16" else np.float32
    in_maps = []
    for c in range(NCORES):
        b, hg = c // CPB, c % CPB
        r = slice(hg * DL, (hg + 1) * DL)
        wq = qkv_w[0 * D :][r, :D]
        wk = qkv_w[1 * D :][r, :D]
        wv = qkv_w[2 * D :][r, :D]
        in_maps.append(
            {
                "xt": np.ascontiguousarray(x[b].T).astype(md_np),
                "wqt": np.ascontiguousarray(wq.T).astype(md_np),
                "wkt": np.ascontiguousarray(wk.T).astype(md_np),
                "wvt": np.ascontiguousarray(wv.T).astype(md_np),
                "wot": None,  # filled below
                "bq": np.ascontiguousarray(
                    qkv_b[0 * D :][r].reshape(2, 128).T
                ).astype(np.float32),
                "bk": np.ascontiguousarray(
                    qkv_b[1 * D :][r].reshape(2, 128).T
                ).astype(np.float32),
                "bv": qkv_b[2 * D :][r].reshape(1, DL).astype(md_np),
            }
        )
    return in_maps


_NC = None


def run(x, qkv_w, qkv_b, out_w, out_b, trace=False, trace_cores=None):
    """Returns (y_full, BassKernelResults)."""
    global _NC
    import ml_dtypes

    mode = MODE
    md_np = ml_dtypes.bfloat16 if mode == "bf16" else np.float32

    x = np.asarray(x, dtype=np.float32)
    qkv_w = np.asarray(qkv_w, dtype=np.float32)
    qkv_b = np.asarray(qkv_b, dtype=np.float32)
    out_w = np.asarray(out_w, dtype=np.float32)
    out_b = np.asarray(out_b, dtype=np.float32)

    in_maps = make_in_maps(x, qkv_w, qkv_b, mode)
    for c in range(NCORES):
        hg = c % CPB
        r = slice(hg * DL, (hg + 1) * DL)
        in_maps[c]["wot"] = np.ascontiguousarray(out_w[:, r].T).astype(md_np)

    if _NC is None:
        _NC = build_nc(mode)
    res = run_bass_kernel_spmd(
        _NC,
        in_maps,
        core_ids=list(range(NCORES)),
        trace=trace,
        trace_cores=trace_cores,
    )

    y = np.zeros((B, N, D), dtype=np.float32)
    for c in range(NCORES):
        y[c // CPB] += res.results[c]["y"]
    y += out_b[None, None, :]
    return y, res


def kernel(x, qkv_w, qkv_b, out_w, out_b):
    return run(x, qkv_w, qkv_b, out_w, out_b)[0]


# revision 27
# speedup vs baseline: 1.1792x; 1.1792x over previous
"""Multi-head attention (B=2, N=2048, D=1024, H=16) on 8 NeuronCores.

Sharding: data-parallel over batch (4 cores per batch element), tensor-
parallel over heads (4 heads per core). Each core computes, for its
(batch, head-group):
    QT/KT = (x Wq^T + bq)^T, V = x Wv^T + bv          (QKV projection)
    S^T   = K Q^T  (per head, k on partitions, q on free)
    P^T   = exp(S^T / 8)                               (no max-subtract:
            scores are O(+-20), exp is safe in fp32)
    [O^T; r] = [V | 1]^T-augmented PV matmul           (row 64 = softmax
            denominators, accumulated alongside O^T)
    O^T  *= 1/r  (per-q normalize)
    Ypart = O @ Wo_slice^T                             (partial over heads)
Host sums the 4 partials per batch and adds out_b.

All weights/x are pre-transposed on the host so the device never
transposes anything.
"""

import numpy as np

import concourse.bass as bass
import concourse.tile as tile
from concourse import bacc, mybir
from concourse.bass_utils import run_bass_kernel_spmd

B, N, D = 2, 2048, 1024
H, HD = 16, 64
NCORES = 8
CPB = 4            # cores per batch element
HPC = H // CPB     # heads per core = 4
DL = HPC * HD      # local (per-core) model dim = 256
FP32 = mybir.dt.float32

# matmul input mode: "f32" | "f32r" | "bf16"
MODE = "f32r"


def _md(mode):
    return {
        "f32": mybir.dt.float32,
        "f32r": mybir.dt.float32r,
        "bf16": mybir.dt.bfloat16,
    }[mode]


def build_nc(mode=MODE):
    md_store = _md(mode)

    nc = bacc.Bacc(
        "TRN2", target_bir_lowering=False, debug=False, num_devices=NCORES
    )

    xt_d = nc.dram_tensor("xt", [D, N], md_store, kind="ExternalInput").ap()
    wqt_d = nc.dram_tensor("wqt", [D, DL], md_store, kind="ExternalInput").ap()
    wkt_d = nc.dram_tensor("wkt", [D, DL], md_store, kind="ExternalInput").ap()
    wvt_d = nc.dram_tensor("wvt", [D, DL], md_store, kind="ExternalInput").ap()
    wot_d = nc.dram_tensor("wot", [DL, D], md_store, kind="ExternalInput").ap()
    bq_d = nc.dram_tensor("bq", [128, 2], FP32, kind="ExternalInput").ap()
    bk_d = nc.dram_tensor("bk", [128, 2], FP32, kind="ExternalInput").ap()
    bv_d = nc.dram_tensor("bv", [1, DL], md_store, kind="ExternalInput").ap()
    y_d = nc.dram_tensor("y", [N, D], FP32, kind="ExternalOutput").ap()

    with tile.TileContext(nc) as tc:
        with (
            tc.tile_pool(name="const", bufs=1) as const,
            tc.tile_pool(name="pt", bufs=2 if mode != "bf16" else 4) as ptp,
            tc.tile_pool(name="ysb", bufs=2) as yp,
            tc.tile_pool(name="small", bufs=2) as smallp,
            tc.tile_pool(name="sps", bufs=2, space="PSUM") as spsum,
            tc.tile_pool(name="ops", bufs=4, space="PSUM") as opsum,
        ):
            # ---- resident SBUF tensors ----
            xt_sb = const.tile([128, 8, N], md_store)
            wqt_sb = const.tile([128, 8, DL], md_store)
            wkt_sb = const.tile([128, 8, DL], md_store)
            wvt_sb = const.tile([128, 8, DL], md_store)
            wot_sb = const.tile([128, 2, D], md_store)
            bq_sb = const.tile([128, 2], FP32)
            bk_sb = const.tile([128, 2], FP32)
            bv_sb = const.tile([1, DL], md_store)
            ones_sb = const.tile([1, 128], md_store)
            ones64 = const.tile([128, HD], FP32)
            qt_sb = const.tile([128, 2, N], md_store)
            kt_sb = const.tile([128, 2, N], md_store)
            v_sb = const.tile([128, 16, HPC, HD + 1], md_store)
            ot_sb = const.tile([128, 2, N], md_store)
            # Softmax denominator rows parked at partition 32*h (DVE APs may
            # only start at partition multiples of 32), q-block along free.
            r_all = const.tile([128, N], FP32)

            nc.sync.dma_start(xt_sb[:], xt_d.rearrange("(o p) n -> p o n", p=128))
            nc.sync.dma_start(wqt_sb[:], wqt_d.rearrange("(o p) m -> p o m", p=128))
            nc.sync.dma_start(wkt_sb[:], wkt_d.rearrange("(o p) m -> p o m", p=128))
            nc.sync.dma_start(wvt_sb[:], wvt_d.rearrange("(o p) m -> p o m", p=128))
            nc.sync.dma_start(wot_sb[:], wot_d.rearrange("(o p) m -> p o m", p=128))
            nc.sync.dma_start(bq_sb[:], bq_d)
            nc.sync.dma_start(bk_sb[:], bk_d)
            nc.sync.dma_start(bv_sb[:], bv_d)
            nc.vector.memset(r_all[:], 1.0)
            nc.vector.memset(ones64[:], 1.0)
            if mode == "f32r":
                # memset doesn't pass the ISA check for float32r; write the
                # 1.0f bit pattern through a uint32 view instead.
                nc.vector.memset(ones_sb[:].bitcast(mybir.dt.uint32), 0x3F800000)
                nc.vector.memset(
                    v_sb[:, :, :, HD : HD + 1].bitcast(mybir.dt.uint32), 0x3F800000
                )
            else:
                nc.vector.memset(ones_sb[:], 1.0)
                nc.vector.memset(v_sb[:, :, :, HD : HD + 1], 1.0)

            Ident = mybir.ActivationFunctionType.Identity

            # ---- QKV projection ----
            # QT/KT: [dl on partitions (2 groups), n on free]
            for w_sb, b_sb, dst in ((wqt_sb, bq_sb, qt_sb), (wkt_sb, bk_sb, kt_sb)):
                for g in range(2):
                    for qs in range(4):
                        ps = spsum.tile([128, 1024], FP32, tag="sps")
                        acc = ps[:, :512]
                        for cc in range(8):
                            nc.tensor.matmul(
                                acc,
                                lhsT=(w_sb[:, cc, g * 128 : (g + 1) * 128]),
                                rhs=(xt_sb[:, cc, qs * 512 : (qs + 1) * 512]),
                                start=(cc == 0),
                                stop=(cc == 7),
                            )
                        nc.scalar.activation(
                            out=dst[:, g, qs * 512 : (qs + 1) * 512],
                            in_=acc,
                            func=Ident,
                            bias=b_sb[:, g : g + 1],
                        )
            # V: natural [n on partitions, dl on free], bias via ones-row matmul
            for nt in range(16):
                ps = spsum.tile([128, 1024], FP32, tag="sps")
                acc = ps[:, :DL]
                for cc in range(8):
                    nc.tensor.matmul(
                        acc,
                        lhsT=(xt_sb[:, cc, nt * 128 : (nt + 1) * 128]),
                        rhs=(wvt_sb[:, cc, :]),
                        start=(cc == 0),
                        stop=False,
                    )
                nc.tensor.matmul(
                    acc,
                    lhsT=ones_sb[:],
                    rhs=bv_sb[:],
                    start=False,
                    stop=True,
                )
                for h in range(HPC):
                    nc.vector.tensor_copy(
                        out=v_sb[:, nt, h, 0:HD], in_=acc[:, h * HD : (h + 1) * HD]
                    )

            # ---- attention, head PAIRS interleaved ----
            # The two heads of a pair have K=64 contractions at partition
            # bases 0 and 64: their S^T matmuls land in different PE row
            # groups and run concurrently, keeping the full array active
            # (a half-idle array trips the HAM throttle to 1.2 GHz).
            for qh in range(2):
                for g in range(2):
                    o_ps = [
                        opsum.tile([128, 512], FP32, tag="ops", name=f"ops_{g}_{qh}_{i}")
                        for i in range(4)  # (h0,j0) (h1,j0) (h0,j1) (h1,j1)
                    ]
                    for kt in range(16):
                        pt = ptp.tile([128, N], md_store, tag="pt")
                        s_pair = [
                            spsum.tile([128, 1024], FP32, tag="sps", name=f"s_{g}_{qh}_{hh}")
                            for hh in range(2)
                        ]
                        for j in range(2):
                            for hh in range(2):
                                po = hh * HD
                                qs = qh * 1024 + j * 512
                                nc.tensor.matmul(
                                    s_pair[hh][:, j * 512 : (j + 1) * 512],
                                    lhsT=kt_sb[
                                        po : po + HD, g, kt * 128 : (kt + 1) * 128
                                    ],
                                    rhs=(qt_sb[po : po + HD, g, qs : qs + 512]),
                                    start=True,
                                    stop=True,
                                )
                        for hh in range(2):
                            nc.scalar.activation(
                                out=pt[:, hh * 1024 : (hh + 1) * 1024],
                                in_=s_pair[hh][:],
                                func=mybir.ActivationFunctionType.Exp,
                                scale=0.125,
                            )
                        for j in range(2):
                            for hh in range(2):
                                h = 2 * g + hh
                                nc.tensor.matmul(
                                    o_ps[2 * j + hh][:65, :],
                                    lhsT=(v_sb[:, kt, h, :]),
                                    rhs=(pt[:, hh * 1024 + j * 512 :][:, :512]),
                                    start=(kt == 0),
                                    stop=(kt == 15),
                                )
                    # Drain accumulators fast (unnormalized) so the next
                    # pass can reuse the PSUM banks; normalization batched.
                    for j in range(2):
                        for hh in range(2):
                            h = 2 * g + hh
                            po = hh * HD
                            qg = qh * 2 + j
                            nc.vector.tensor_copy(
                                out=r_all[
                                    32 * h : 32 * h + 1, qg * 512 : (qg + 1) * 512
                                ],
                                in_=o_ps[2 * j + hh][64:65, :],
                            )
                            nc.vector.tensor_copy(
                                out=ot_sb[po : po + HD, g, qg * 512 : (qg + 1) * 512],
                                in_=o_ps[2 * j + hh][0:HD, :],
                            )

                # ---- normalize + output-projection for this q-half ----
                # Runs as soon as both head-pairs of the q-half have drained,
                # overlapping the next q-half's attention on PE/DVE bubbles.
                nc.vector.reciprocal(
                    r_all[:, qh * 1024 : (qh + 1) * 1024],
                    r_all[:, qh * 1024 : (qh + 1) * 1024],
                )
                for h in range(HPC):
                    g, po = h // 2, (h % 2) * HD
                    for j in range(2):
                        qg = qh * 2 + j
                        # Broadcast the reciprocal row across 64 partitions on
                        # the PE (ones[1,64].T @ r[1,512]); gpsimd
                        # partition_broadcast mis-reads nonzero base
                        # partitions on HW.
                        bc_ps = opsum.tile([128, 512], FP32, tag="ops")
                        nc.tensor.matmul(
                            bc_ps[:HD, :],
                            lhsT=ones64[32 * h : 32 * h + 1, :],
                            rhs=r_all[32 * h : 32 * h + 1, qg * 512 : (qg + 1) * 512],
                            start=True,
                            stop=True,
                            # auto-derive caps at base partition 64
                            tile_position=(32 * h, 0) if h == 3 else None,
                        )
                        sl = ot_sb[po : po + HD, g, qg * 512 : (qg + 1) * 512]
                        nc.vector.tensor_tensor(
                            out=sl,
                            in0=sl,
                            in1=bc_ps[0:HD, :],
                            op=mybir.AluOpType.mult,
                        )
                for nt in range(qh * 8, qh * 8 + 8):
                    y_sb = yp.tile([128, D], FP32, tag="y")
                    for jg in range(2):
                        yps = opsum.tile([128, 512], FP32, tag="ops")
                        for dg in range(2):
                            nc.tensor.matmul(
                                yps[:],
                                lhsT=(ot_sb[:, dg, nt * 128 : (nt + 1) * 128]),
                                rhs=(wot_sb[:, dg, jg * 512 : (jg + 1) * 512]),
                                start=(dg == 0),
                                stop=(dg == 1),
                            )
                        nc.vector.tensor_copy(
                            out=y_sb[:, jg * 512 : (jg + 1) * 512], in_=yps[:]
                        )
                    nc.sync.dma_start(y_d[nt * 128 : (nt + 1) * 128, :], y_sb[:])

    nc.compile()
    return nc


def make_in_maps(x, qkv_w, qkv_b, mode=MODE):
    """Host-side shard + pre-transpose. Core c -> batch c//CPB, heads
    [(c%CPB)*HPC, ...+HPC)."""
    import ml_dtypes

    md_np = ml_dtypes.bfloat16 if mode == "bf16" else np.float32
    in_maps = []
    for c in range(NCORES):
        b, hg = c // CPB, c % CPB
        r = slice(hg * DL, (hg + 1) * DL)
        wq = qkv_w[0 * D :][r, :D]
        wk = qkv_w[1 * D :][r, :D]
        wv = qkv_w[2 * D :][r, :D]
        in_maps.append(
            {
                "xt": np.ascontiguousarray(x[b].T).astype(md_np),
                "wqt": np.ascontiguousarray(wq.T).astype(md_np),
                "wkt": np.ascontiguousarray(wk.T).astype(md_np),
                "wvt": np.ascontiguousarray(wv.T).astype(md_np),
                "wot": None,  # filled below
                "bq": np.ascontiguousarray(
                    qkv_b[0 * D :][r].reshape(2, 128).T
                ).astype(np.float32),
                "bk": np.ascontiguousarray(
                    qkv_b[1 * D :][r].reshape(2, 128).T
                ).astype(np.float32),
                "bv": qkv_b[2 * D :][r].reshape(1, DL).astype(md_np),
            }
        )
    return in_maps


_NC = None


def run(x, qkv_w, qkv_b, out_w, out_b, trace=False, trace_cores=None):
    """Returns (y_full, BassKernelResults)."""
    global _NC
    import ml_dtypes

    mode = MODE
    md_np = ml_dtypes.bfloat16 if mode == "bf16" else np.float32

    x = np.asarray(x, dtype=np.float32)
    qkv_w = np.asarray(qkv_w, dtype=np.float32)
    qkv_b = np.asarray(qkv_b, dtype=np.float32)
    out_w = np.asarray(out_w, dtype=np.float32)
    out_b = np.asarray(out_b, dtype=np.float32)

    in_maps = make_in_maps(x, qkv_w, qkv_b, mode)
    for c in range(NCORES):
        hg = c % CPB
        r = slice(hg * DL, (hg + 1) * DL)
        in_maps[c]["wot"] = np.ascontiguousarray(out_w[:, r].T).astype(md_np)

    if _NC is None:
        _NC = build_nc(mode)
    res = run_bass_kernel_spmd(
        _NC,
        in_maps,
        core_ids=list(range(NCORES)),
        trace=trace,
        trace_cores=trace_cores,
    )

    y = np.zeros((B, N, D), dtype=np.float32)
    for c in range(NCORES):
        y[c // CPB] += res.results[c]["y"]
    y += out_b[None, None, :]
    return y, res


def kernel(x, qkv_w, qkv_b, out_w, out_b):
    return run(x, qkv_w, qkv_b, out_w, out_b)[0]


# revision 28
# speedup vs baseline: 1.5569x; 1.3203x over previous
"""Multi-head attention (B=2, N=2048, D=1024, H=16) on 8 NeuronCores.

Sharding: data-parallel over batch (4 cores per batch element), tensor-
parallel over heads (4 heads per core). Each core computes, for its
(batch, head-group):
    QT/KT = (x Wq^T + bq)^T, V = x Wv^T + bv          (QKV projection)
    S^T   = K Q^T  (per head, k on partitions, q on free)
    P^T   = exp(S^T / 8)                               (no max-subtract:
            scores are O(+-20), exp is safe in fp32)
    [O^T; r] = [V | 1]^T-augmented PV matmul           (row 64 = softmax
            denominators, accumulated alongside O^T)
    O^T  *= 1/r  (per-q normalize)
    Ypart = O @ Wo_slice^T                             (partial over heads)
Host sums the 4 partials per batch and adds out_b.

All weights/x are pre-transposed on the host so the device never
transposes anything.
"""

import numpy as np

import concourse.bass as bass
import concourse.tile as tile
from concourse import bacc, mybir
from concourse.bass_utils import run_bass_kernel_spmd

B, N, D = 2, 2048, 1024
H, HD = 16, 64
NCORES = 8
CPB = 4            # cores per batch element
HPC = H // CPB     # heads per core = 4
DL = HPC * HD      # local (per-core) model dim = 256
FP32 = mybir.dt.float32

# matmul input mode: "f32" | "f32r" | "bf16"
MODE = "bf16"


def _md(mode):
    return {
        "f32": mybir.dt.float32,
        "f32r": mybir.dt.float32r,
        "bf16": mybir.dt.bfloat16,
    }[mode]


def build_nc(mode=MODE):
    md_store = _md(mode)

    nc = bacc.Bacc(
        "TRN2", target_bir_lowering=False, debug=False, num_devices=NCORES
    )

    xt_d = nc.dram_tensor("xt", [D, N], md_store, kind="ExternalInput").ap()
    wqt_d = nc.dram_tensor("wqt", [D, DL], md_store, kind="ExternalInput").ap()
    wkt_d = nc.dram_tensor("wkt", [D, DL], md_store, kind="ExternalInput").ap()
    wvt_d = nc.dram_tensor("wvt", [D, DL], md_store, kind="ExternalInput").ap()
    wot_d = nc.dram_tensor("wot", [DL, D], md_store, kind="ExternalInput").ap()
    bq_d = nc.dram_tensor("bq", [128, 2], FP32, kind="ExternalInput").ap()
    bk_d = nc.dram_tensor("bk", [128, 2], FP32, kind="ExternalInput").ap()
    bv_d = nc.dram_tensor("bv", [1, DL], md_store, kind="ExternalInput").ap()
    y_d = nc.dram_tensor("y", [N, D], FP32, kind="ExternalOutput").ap()

    with tile.TileContext(nc) as tc:
        with (
            tc.tile_pool(name="const", bufs=1) as const,
            tc.tile_pool(name="pt", bufs=2 if mode != "bf16" else 4) as ptp,
            tc.tile_pool(name="ysb", bufs=2) as yp,
            tc.tile_pool(name="small", bufs=2) as smallp,
            tc.tile_pool(name="sps", bufs=2, space="PSUM") as spsum,
            tc.tile_pool(name="ops", bufs=4, space="PSUM") as opsum,
        ):
            # ---- resident SBUF tensors ----
            xt_sb = const.tile([128, 8, N], md_store)
            wqt_sb = const.tile([128, 8, DL], md_store)
            wkt_sb = const.tile([128, 8, DL], md_store)
            wvt_sb = const.tile([128, 8, DL], md_store)
            wot_sb = const.tile([128, 2, D], md_store)
            bq_sb = const.tile([128, 2], FP32)
            bk_sb = const.tile([128, 2], FP32)
            bv_sb = const.tile([1, DL], md_store)
            ones_sb = const.tile([1, 128], md_store)
            ones64 = const.tile([128, HD], FP32)
            qt_sb = const.tile([128, 2, N], md_store)
            kt_sb = const.tile([128, 2, N], md_store)
            v_sb = const.tile([128, 16, HPC, HD + 1], md_store)
            ot_sb = const.tile([128, 2, N], md_store)
            # Softmax denominator rows parked at partition 32*h (DVE APs may
            # only start at partition multiples of 32), q-block along free.
            r_all = const.tile([128, N], FP32)

            nc.sync.dma_start(xt_sb[:], xt_d.rearrange("(o p) n -> p o n", p=128))
            nc.sync.dma_start(wqt_sb[:], wqt_d.rearrange("(o p) m -> p o m", p=128))
            nc.sync.dma_start(wkt_sb[:], wkt_d.rearrange("(o p) m -> p o m", p=128))
            nc.sync.dma_start(wvt_sb[:], wvt_d.rearrange("(o p) m -> p o m", p=128))
            nc.sync.dma_start(wot_sb[:], wot_d.rearrange("(o p) m -> p o m", p=128))
            nc.sync.dma_start(bq_sb[:], bq_d)
            nc.sync.dma_start(bk_sb[:], bk_d)
            nc.sync.dma_start(bv_sb[:], bv_d)
            nc.vector.memset(r_all[:], 1.0)
            nc.vector.memset(ones64[:], 1.0)
            if mode == "f32r":
                # memset doesn't pass the ISA check for float32r; write the
                # 1.0f bit pattern through a uint32 view instead.
                nc.vector.memset(ones_sb[:].bitcast(mybir.dt.uint32), 0x3F800000)
                nc.vector.memset(
                    v_sb[:, :, :, HD : HD + 1].bitcast(mybir.dt.uint32), 0x3F800000
                )
            else:
                nc.vector.memset(ones_sb[:], 1.0)
                nc.vector.memset(v_sb[:, :, :, HD : HD + 1], 1.0)

            Ident = mybir.ActivationFunctionType.Identity

            # ---- QKV projection ----
            # QT/KT: [dl on partitions (2 groups), n on free]
            for w_sb, b_sb, dst in ((wqt_sb, bq_sb, qt_sb), (wkt_sb, bk_sb, kt_sb)):
                for g in range(2):
                    for qs in range(4):
                        ps = spsum.tile([128, 1024], FP32, tag="sps")
                        acc = ps[:, :512]
                        for cc in range(8):
                            nc.tensor.matmul(
                                acc,
                                lhsT=(w_sb[:, cc, g * 128 : (g + 1) * 128]),
                                rhs=(xt_sb[:, cc, qs * 512 : (qs + 1) * 512]),
                                start=(cc == 0),
                                stop=(cc == 7),
                            )
                        nc.scalar.activation(
                            out=dst[:, g, qs * 512 : (qs + 1) * 512],
                            in_=acc,
                            func=Ident,
                            bias=b_sb[:, g : g + 1],
                        )
            # V: natural [n on partitions, dl on free], bias via ones-row matmul
            for nt in range(16):
                ps = spsum.tile([128, 1024], FP32, tag="sps")
                acc = ps[:, :DL]
                for cc in range(8):
                    nc.tensor.matmul(
                        acc,
                        lhsT=(xt_sb[:, cc, nt * 128 : (nt + 1) * 128]),
                        rhs=(wvt_sb[:, cc, :]),
                        start=(cc == 0),
                        stop=False,
                    )
                nc.tensor.matmul(
                    acc,
                    lhsT=ones_sb[:],
                    rhs=bv_sb[:],
                    start=False,
                    stop=True,
                )
                for h in range(HPC):
                    nc.vector.tensor_copy(
                        out=v_sb[:, nt, h, 0:HD], in_=acc[:, h * HD : (h + 1) * HD]
                    )

            # ---- attention, head PAIRS interleaved ----
            # The two heads of a pair have K=64 contractions at partition
            # bases 0 and 64: their S^T matmuls land in different PE row
            # groups and run concurrently, keeping the full array active
            # (a half-idle array trips the HAM throttle to 1.2 GHz).
            for qh in range(2):
                for g in range(2):
                    o_ps = [
                        opsum.tile([128, 512], FP32, tag="ops", name=f"ops_{g}_{qh}_{i}")
                        for i in range(4)  # (h0,j0) (h1,j0) (h0,j1) (h1,j1)
                    ]
                    for kt in range(16):
                        pt = ptp.tile([128, N], md_store, tag="pt")
                        s_pair = [
                            spsum.tile([128, 1024], FP32, tag="sps", name=f"s_{g}_{qh}_{hh}")
                            for hh in range(2)
                        ]
                        for j in range(2):
                            for hh in range(2):
                                po = hh * HD
                                qs = qh * 1024 + j * 512
                                nc.tensor.matmul(
                                    s_pair[hh][:, j * 512 : (j + 1) * 512],
                                    lhsT=kt_sb[
                                        po : po + HD, g, kt * 128 : (kt + 1) * 128
                                    ],
                                    rhs=(qt_sb[po : po + HD, g, qs : qs + 512]),
                                    start=True,
                                    stop=True,
                                )
                        for hh in range(2):
                            nc.scalar.activation(
                                out=pt[:, hh * 1024 : (hh + 1) * 1024],
                                in_=s_pair[hh][:],
                                func=mybir.ActivationFunctionType.Exp,
                                scale=0.125,
                            )
                        for j in range(2):
                            for hh in range(2):
                                h = 2 * g + hh
                                nc.tensor.matmul(
                                    o_ps[2 * j + hh][:65, :],
                                    lhsT=(v_sb[:, kt, h, :]),
                                    rhs=(pt[:, hh * 1024 + j * 512 :][:, :512]),
                                    start=(kt == 0),
                                    stop=(kt == 15),
                                )
                    # Drain accumulators fast (unnormalized) so the next
                    # pass can reuse the PSUM banks; normalization batched.
                    for j in range(2):
                        for hh in range(2):
                            h = 2 * g + hh
                            po = hh * HD
                            qg = qh * 2 + j
                            nc.vector.tensor_copy(
                                out=r_all[
                                    32 * h : 32 * h + 1, qg * 512 : (qg + 1) * 512
                                ],
                                in_=o_ps[2 * j + hh][64:65, :],
                            )
                            nc.vector.tensor_copy(
                                out=ot_sb[po : po + HD, g, qg * 512 : (qg + 1) * 512],
                                in_=o_ps[2 * j + hh][0:HD, :],
                            )

                # ---- normalize + output-projection for this q-half ----
                # Runs as soon as both head-pairs of the q-half have drained,
                # overlapping the next q-half's attention on PE/DVE bubbles.
                nc.vector.reciprocal(
                    r_all[:, qh * 1024 : (qh + 1) * 1024],
                    r_all[:, qh * 1024 : (qh + 1) * 1024],
                )
                for h in range(HPC):
                    g, po = h // 2, (h % 2) * HD
                    for j in range(2):
                        qg = qh * 2 + j
                        # Broadcast the reciprocal row across 64 partitions on
                        # the PE (ones[1,64].T @ r[1,512]); gpsimd
                        # partition_broadcast mis-reads nonzero base
                        # partitions on HW.
                        bc_ps = opsum.tile([128, 512], FP32, tag="ops")
                        nc.tensor.matmul(
                            bc_ps[:HD, :],
                            lhsT=ones64[32 * h : 32 * h + 1, :],
                            rhs=r_all[32 * h : 32 * h + 1, qg * 512 : (qg + 1) * 512],
                            start=True,
                            stop=True,
                            # auto-derive caps at base partition 64
                            tile_position=(32 * h, 0) if h == 3 else None,
                        )
                        sl = ot_sb[po : po + HD, g, qg * 512 : (qg + 1) * 512]
                        nc.vector.tensor_tensor(
                            out=sl,
                            in0=sl,
                            in1=bc_ps[0:HD, :],
                            op=mybir.AluOpType.mult,
                        )
                for nt in range(qh * 8, qh * 8 + 8):
                    y_sb = yp.tile([128, D], FP32, tag="y")
                    for jg in range(2):
                        yps = opsum.tile([128, 512], FP32, tag="ops")
                        for dg in range(2):
                            nc.tensor.matmul(
                                yps[:],
                                lhsT=(ot_sb[:, dg, nt * 128 : (nt + 1) * 128]),
                                rhs=(wot_sb[:, dg, jg * 512 : (jg + 1) * 512]),
                                start=(dg == 0),
                                stop=(dg == 1),
                            )
                        nc.vector.tensor_copy(
                            out=y_sb[:, jg * 512 : (jg + 1) * 512], in_=yps[:]
                        )
                    nc.sync.dma_start(y_d[nt * 128 : (nt + 1) * 128, :], y_sb[:])

    nc.compile()
    return nc


def make_in_maps(x, qkv_w, qkv_b, mode=MODE):
    """Host-side shard + pre-transpose. Core c -> batch c//CPB, heads
    [(c%CPB)*HPC, ...+HPC)."""
    import ml_dtypes

    md_np = ml_dtypes.bfloat16 if mode == "bf16" else np.float32
    in_maps = []
    for c in range(NCORES):
        b, hg = c // CPB, c % CPB
        r = slice(hg * DL, (hg + 1) * DL)
        wq = qkv_w[0 * D :][r, :D]
        wk = qkv_w[1 * D :][r, :D]
        wv = qkv_w[2 * D :][r, :D]
        in_maps.append(
            {
                "xt": np.ascontiguousarray(x[b].T).astype(md_np),
                "wqt": np.ascontiguousarray(wq.T).astype(md_np),
                "wkt": np.ascontiguousarray(wk.T).astype(md_np),
                "wvt": np.ascontiguousarray(wv.T).astype(md_np),
                "wot": None,  # filled below
                "bq": np.ascontiguousarray(
                    qkv_b[0 * D :][r].reshape(2, 128).T
                ).astype(np.float32),
                "bk": np.ascontiguousarray(
                    qkv_b[1 * D :][r].reshape(2, 128).T
                ).astype(np.float32),
                "bv": qkv_b[2 * D :][r].reshape(1, DL).astype(md_np),
            }
        )
    return in_maps


_NC = None


def run(x, qkv_w, qkv_b, out_w, out_b, trace=False, trace_cores=None):
    """Returns (y_full, BassKernelResults)."""
    global _NC
    import ml_dtypes

    mode = MODE
    md_np = ml_dtypes.bfloat16 if mode == "bf16" else np.float32

    x = np.asarray(x, dtype=np.float32)
    qkv_w = np.asarray(qkv_w, dtype=np.float32)
    qkv_b = np.asarray(qkv_b, dtype=np.float32)
    out_w = np.asarray(out_w, dtype=np.float32)
    out_b = np.asarray(out_b, dtype=np.float32)

    in_maps = make_in_maps(x, qkv_w, qkv_b, mode)
    for c in range(NCORES):
        hg = c % CPB
        r = slice(hg * DL, (hg + 1) * DL)
        in_maps[c]["wot"] = np.ascontiguousarray(out_w[:, r].T).astype(md_np)

    if _NC is None:
        _NC = build_nc(mode)
    res = run_bass_kernel_spmd(
        _NC,
        in_maps,
        core_ids=list(range(NCORES)),
        trace=trace,
        trace_cores=trace_cores,
    )

    y = np.zeros((B, N, D), dtype=np.float32)
    for c in range(NCORES):
        y[c // CPB] += res.results[c]["y"]
    y += out_b[None, None, :]
    return y, res


def kernel(x, qkv_w, qkv_b, out_w, out_b):
    return run(x, qkv_w, qkv_b, out_w, out_b)[0]
